# revision 41
# baseline (speedup 1.0000x reference)
"""Self-contained Trainium2 Bass kernel for nn_AttentionBlock (GroupNorm +
single-head attention + residual).

Reference computation (shapes hardcoded):
    x: [B=4, H=64, W=64, C=256] f32
    xn = GroupNorm(x, groups=8, eps=1e-3) * gamma + beta
    q/k/v = xn @ W{q,k,v} + b{q,k,v}           # per batch, N=H*W=4096 tokens
    attn = softmax(q @ k^T / sqrt(C))
    out  = xn + (attn @ v) @ Wp + bp

Sharding: 8 cores = (batch b, query-half h). Each core receives its batch's
full token sequence (channel-major / pre-transposed on the host) with rows
rotated so its 2048 query rows come first; it computes groupnorm + K/V for
all 4096 tokens and attention for its 2048 query rows. No collectives. The
host reassembles the 8 transposed [2,128,2048] outputs.

Key layout choices:
  - x arrives channel-major [cb, p, n] so no on-device input transposes.
  - q/k/v projections + scores + attn@v run in fp8 with DoubleRow (2x).
  - exp'd scores stay key-major (eT); attn@v uses the ones-column trick for
    softmax row sums; all biases are folded into existing copies/residual.
  - output leaves the device transposed [cob, p, n]; host de-transposes.
"""

import numpy as np

import concourse.bass as bass
import concourse.tile as tile
from concourse import mybir
from concourse.tile import ScopedClock

# Problem shapes (hardcoded per contract)
B, H, W, C = 4, 64, 64, 256
N = H * W            # 4096 tokens per batch image
NQ = N // 2          # 2048 query rows per core
G = 8                # groupnorm groups
CG = C // G          # 32 channels per group
EPS = 1e-3
P = 128
FD = 512             # matmul moving-operand free dim (one PSUM bank of f32)
NKB = N // P         # 32 key blocks
NQT = NQ // FD       # 4 query tiles per core
SCALE = float(C) ** -0.5
F32 = mybir.dt.float32
BF16 = mybir.dt.bfloat16
FP8 = mybir.dt.float8e4
AF = mybir.ActivationFunctionType
DR = mybir.MatmulPerfMode.DoubleRow

# dev knob: repeat the whole body R times inside one NEFF (throughput probe)
REPS = 1
LOOPN = 1
VARIANT = "full"


def _drain_and_barrier_split(self, tick_clock, wait_clock):
    """Replacement for TileContext._drain_and_barrier.

    The walrus build in this container rejects sem waits on InstDrain (and
    >1 wait on a NOP), so carry the end-of-kernel waits on a chain of NOPs
    with one wait each, drain without sync, and use the sem-only (no-Drain)
    all-engine barrier around semaphore cleanup.
    """
    nc = self.nc
    carrier = nc.sync.nop(nofuse=True)
    wait_clock.add_sem_waits(
        carrier.ins, ScopedClock({None: tick_clock.global_clock})
    )
    si = carrier.ins.sync_info
    waits = list(si.on_wait) if si is not None and si.on_wait else []
    if len(waits) > 1:
        carrier.ins.sync_info = mybir.SyncInfo(
            on_wait=waits[:1], on_update=list(si.on_update or [])
        )
        for w in waits[1:]:
            extra = nc.sync.nop(nofuse=True)
            extra.ins.sync_info = mybir.SyncInfo(on_wait=[w], on_update=[])
    nc.sync.drain()
    nc.all_engine_barrier(sem_only=True)
    assert self.sems is not None
    popped = nc._tile_sem_poison_stack.pop()
    assert popped is self._sem_poison
    nc.clear_and_free_semaphores(list(self.sems.allocated().values()))
    nc.all_engine_barrier(sem_only=True)


tile.TileContext._drain_and_barrier = _drain_and_barrier_split

_wsplit_ctr = 0


def _split_multi_waits(nc: bass.Bass):
    """Walrus in this container supports at most one sync wait per
    instruction (and none on Drain). Hoist excess waits onto NoOps placed
    just before the instruction on the same engine — sequencers process
    instructions in order, so blocking on the NoOp is equivalent."""
    global _wsplit_ctr
    for f in nc.m.functions:
        for bb in f.blocks:
            new_insts = []
            for ins in bb.instructions:
                si = getattr(ins, "sync_info", None)
                waits = list(si.on_wait) if si is not None and si.on_wait else []
                limit = 0 if ins.opcode == "Drain" else 1
                if len(waits) > limit:
                    keep = waits[len(waits) - limit:] if limit else []
                    hoist = waits[: len(waits) - limit]
                    for w in hoist:
                        _wsplit_ctr += 1
                        nop = mybir.InstNoOp(
                            name=f"I-wsplit-{_wsplit_ctr}",
                            engine=ins.engine,
                            sync_info=mybir.SyncInfo(on_wait=[w], on_update=[]),
                        )
                        new_insts.append(nop)
                    ins.sync_info = mybir.SyncInfo(
                        on_wait=keep, on_update=list(si.on_update or [])
                    )
                new_insts.append(ins)
            bb.instructions[:] = new_insts


# ---- single-blob input packing (one input param + one output param:
# each extra parameter costs ~2 ms/execution in this PJRT path) ----
_SEGS = [
    ("x", N * C),        # channel-major: [cb, p, n]
    ("wq", C * C), ("wk", C * C), ("wv", C * C), ("wp", C * C),
    ("bq", C), ("bk", C), ("bv", C), ("bp", C),
    ("gamma", C), ("beta", C),
    ("ident", P * P),
    ("egrp", P * 2 * G),
    ("egrpt", G * C),
]
_OFF = {}
_total = 0
for _nm, _sz in _SEGS:
    _OFF[_nm] = _total
    _total += _sz
BLOB_SIZE = _total


def build_nc(split_waits: bool = True) -> bass.Bass:
    nc = bass.Bass(enable_partition_id=False)
    blob = nc.declare_dram_parameter("blob", [BLOB_SIZE], F32, isOutput=False)[:]

    def seg(name, size):
        return blob[_OFF[name]:_OFF[name] + size]

    x = seg("x", N * C).bitcast(BF16)[0:N * C].rearrange(
        "(cb p n) -> p cb n", cb=2, p=P, n=N)
    wq = seg("wq", C * C).rearrange("(ci co) -> ci co", co=C)
    wk = seg("wk", C * C).rearrange("(ci co) -> ci co", co=C)
    wv = seg("wv", C * C).rearrange("(ci co) -> ci co", co=C)
    wp = seg("wp", C * C).rearrange("(ci co) -> ci co", co=C)
    bq = seg("bq", C)
    b6seg = seg("bq", 6 * C)
    bk = seg("bk", C)
    bv = seg("bv", C)
    bp = seg("bp", C)
    gamma = seg("gamma", C)
    beta = seg("beta", C)
    ident = seg("ident", P * P).rearrange("(a b) -> a b", b=P)
    egrp = seg("egrp", P * 2 * G).rearrange("(a b) -> a b", b=2 * G)
    egrpt = seg("egrpt", G * C).rearrange("(a b) -> a b", b=C)
    # transposed output: out[cob, p, n] = channel cob*128+p of query row n
    out = nc.declare_dram_parameter("out", [2, P, NQ], F32, isOutput=True)[:]

    with tile.TileContext(nc) as tc:
        if LOOPN > 1:
            with tc.For_i(0, LOOPN, 1):
                _body(nc, tc, x, wq, wk, wv, wp, b6seg, ident, egrp,
                      egrpt, out)
        else:
            for _rep in range(REPS):
                _body(nc, tc, x, wq, wk, wv, wp, b6seg, ident, egrp,
                      egrpt, out)
    if split_waits:
        _split_multi_waits(nc)
    return nc


def _body(nc, tc, x, wq, wk, wv, wp, b6seg, ident, egrp, egrpt, out):
    from contextlib import ExitStack
    ctx = ExitStack()
    with ctx:
        consts = ctx.enter_context(tc.tile_pool(name="consts", bufs=1))
        big = ctx.enter_context(tc.tile_pool(name="big", bufs=1))
        work = ctx.enter_context(tc.tile_pool(name="work", bufs=3))
        small = ctx.enter_context(tc.tile_pool(name="small", bufs=2))

        # ---- x^T first: 3 big chunks, one per DMA-capable queue (a single
        # queue tops out well below the core's aggregate HBM bandwidth, and
        # each dma_start costs ~1us of dispatch on its host queue) ----
        xT = big.tile([P, 2, N], BF16, tag="xT")      # raw input, ch-major
        NCH = 4
        nch = N // NCH
        _X6 = ((nc.sync, 0, 1024), (nc.scalar, 1536, 2560),
               (nc.gpsimd, 3072, 3584), (nc.sync, 1024, 1536),
               (nc.scalar, 2560, 3072), (nc.gpsimd, 3584, 4096))
        for eng, lo, hi in _X6:
            eng.dma_start(out=xT[:, :, lo:hi], in_=x[:, :, lo:hi])

        # small constants: bq/bk/bv/bp/gamma/beta are consecutive [2,P]
        # blob segments — one batched DMA instead of six dispatches
        b6 = consts.tile([P, 12], F32, tag="b6")
        nc.gpsimd.dma_start(
            out=b6, in_=b6seg.rearrange("(k b p) -> p (k b)", b=2, p=P))
        bq_pp = b6[:, 0:2]
        bk_pp = b6[:, 2:4]
        bv_pp = b6[:, 4:6]
        bp_pp = b6[:, 6:8]
        gamma_pp = b6[:, 8:10]
        beta_pp = b6[:, 10:12]
        egrp_sb = consts.tile([P, 2 * G], F32, tag="egrp")
        nc.gpsimd.dma_start(out=egrp_sb, in_=egrp)
        egrpt_sb = consts.tile([G, C], F32, tag="egrpt")
        nc.gpsimd.dma_start(out=egrpt_sb, in_=egrpt)
        ident_f = consts.tile([P, P], F32, tag="ident_f")
        nc.sync.dma_start(out=ident_f, in_=ident)

        # weights: stage f32 (scalar queue, behind its x chunk); the fp8 /
        # bf16 casts are issued later so the DVE queue serves stats first
        w_stage = {}
        for name, wh in (("q", wq), ("k", wk), ("v", wv), ("p", wp)):
            tf = work.tile([P, 2, C], F32, tag=f"wstage_{name}",
                           name=f"wstage_{name}")
            nc.scalar.dma_start(
                out=tf, in_=wh.rearrange("(kb p) co -> p kb co", p=P))
            w_stage[name] = tf

        # ---- phase A: stats, normalize ----
        xnr = big.tile([P, 2, N], F32, tag="xnr")     # xn + bpc (residual)
        xnb = big.tile([P, 2, N], FP8, tag="xnb")     # xn fp8 for projections

        with tc.tile_pool(name="psA", bufs=1, space="PSUM") as psA, \
             tc.tile_pool(name="psB", bufs=2, space="PSUM") as psB:
            # per-channel mean/var via bn_stats (channels on partitions),
            # chunk-chasing the (3-queue-parallel) x DMA.
            ps_g = psA.tile([G, 2], F32, tag="gstats")
            fmax = nc.vector.BN_STATS_FMAX
            nsub = N // fmax
            stats = [work.tile([P, nsub, nc.vector.BN_STATS_DIM], F32,
                               tag=f"bnstats{cb}", name=f"bnstats{cb}")
                     for cb in range(2)]
            _ARRIVE = (0, 1, 3, 4, 6, 2, 5, 7)  # slice order per _X6
            for si in _ARRIVE:
                for cb in range(2):
                    nc.vector.bn_stats(
                        out=stats[cb][:, si, :],
                        in_=xT[:, cb, si * fmax:(si + 1) * fmax])
            for cb in range(2):
                mv = work.tile([P, nc.vector.BN_AGGR_DIM], F32, tag="bnmv")
                nc.vector.bn_aggr(out=mv, in_=stats[cb])
                # pack (mean_c, E[x^2]_c = var_c + mean_c^2)
                pk = work.tile([P, 2], F32, tag="pk")
                nc.vector.tensor_copy(out=pk[:, 0:1], in_=mv[:, 0:1])
                msq = work.tile([P, 1], F32, tag="msq")
                nc.vector.tensor_mul(out=msq, in0=mv[:, 0:1], in1=mv[:, 0:1])
                nc.vector.tensor_add(out=pk[:, 1:2], in0=mv[:, 1:2], in1=msq)
                nc.tensor.matmul(ps_g, lhsT=egrp_sb[:, cb * G:(cb + 1) * G],
                                 rhs=pk, start=(cb == 0), stop=(cb == 1),
                                 skip_group_check=True)

            # finalize on G partitions: mean_g, rstd_g
            gsb = small.tile([G, 2], F32, tag="gsb")
            nc.vector.tensor_scalar_mul(gsb, ps_g, 1.0 / CG)
            gmean = gsb[:, 0:1]
            gex2 = gsb[:, 1:2]
            gmsq = small.tile([G, 1], F32, tag="gmsq")
            nc.vector.tensor_mul(out=gmsq, in0=gmean, in1=gmean)
            gvar = small.tile([G, 1], F32, tag="gvar")
            nc.vector.tensor_tensor(out=gvar, in0=gex2, in1=gmsq,
                                    op=mybir.AluOpType.subtract)
            eps_sb = small.tile([G, 1], F32, tag="eps")
            nc.vector.memset(eps_sb, EPS)
            gstd = small.tile([G, 1], F32, tag="gstd")
            nc.scalar.activation(out=gstd, in_=gvar, func=AF.Sqrt, bias=eps_sb)
            gpack = small.tile([G, 2], F32, tag="gpack")
            nc.vector.tensor_copy(out=gpack[:, 0:1], in_=gmean)
            nc.vector.reciprocal(out=gpack[:, 1:2], in_=gstd)

            # deferred casts (weights fp8/bf16, identity bf16, bv bf16) —
            # issued after stats so the DVE queue serves stats first
            w_sbs = {}
            for name, dt in (("q", FP8), ("k", FP8), ("v", FP8),
                             ("p", BF16)):
                t = consts.tile([P, 2, C], dt, tag=f"w{name}",
                                name=f"w{name}")
                nc.vector.tensor_copy(out=t, in_=w_stage[name])
                w_sbs[name] = t
            ident_bf = consts.tile([P, P], BF16, tag="ident_bf")
            nc.vector.tensor_copy(out=ident_bf, in_=ident_f)
            bv_bf = consts.tile([P, 2], BF16, tag="bv_bf")
            nc.vector.tensor_copy(out=bv_bf, in_=bv_pp)

            # broadcast group stats back to channel partitions; build
            # all6[:, cb]   = scale_c  = rstd * gamma
            # all6[:, 2+cb] = shift_c  = beta - mean*scale
            # all6[:, 4+cb] = shift_c + bpc_c   (residual version)
            all6 = consts.tile([P, 6], F32, tag="all6")
            for cb in range(2):
                ps_bc = psA.tile([P, 2], F32, tag="bc", name=f"bc{cb}")
                nc.tensor.matmul(ps_bc, lhsT=egrpt_sb[:, cb * P:(cb + 1) * P],
                                 rhs=gpack, start=True, stop=True)
                mr = small.tile([P, 2], F32, tag="mr")
                nc.vector.tensor_copy(out=mr, in_=ps_bc)
                nc.vector.tensor_mul(out=all6[:, cb:cb + 1], in0=mr[:, 1:2],
                                     in1=gamma_pp[:, cb:cb + 1])
                ms = small.tile([P, 1], F32, tag="ms")
                nc.vector.tensor_mul(out=ms, in0=mr[:, 0:1],
                                     in1=all6[:, cb:cb + 1])
                nc.vector.tensor_tensor(out=all6[:, 2 + cb:3 + cb],
                                        in0=beta_pp[:, cb:cb + 1], in1=ms,
                                        op=mybir.AluOpType.subtract)

            # normalize: xnb (fp8, for projections). The n-range that gates
            # the first score groups (h0) runs on gpsimd (cb0) + scalar
            # (cb1) in quarter chunks; the later h1 range follows on
            # gpsimd. The vector queue is left free for the weight casts
            # and k^T copies that also gate the first scores.
            def _xnb(eng, cb, lo, hi):
                if eng is nc.scalar:
                    nc.scalar.activation(
                        out=xnb[:, cb, lo:hi], in_=xT[:, cb, lo:hi],
                        func=AF.Identity, scale=all6[:, cb:cb + 1],
                        bias=all6[:, 2 + cb:3 + cb])
                else:
                    eng.tensor_scalar(
                        out=xnb[:, cb, lo:hi], in0=xT[:, cb, lo:hi],
                        scalar1=all6[:, cb:cb + 1],
                        scalar2=all6[:, 2 + cb:3 + cb],
                        op0=mybir.AluOpType.mult, op1=mybir.AluOpType.add)

            for qtr in range(2):
                lo, hi = qtr * (N // 4), (qtr + 1) * (N // 4)
                _xnb(nc.gpsimd, 0, lo, hi)
                _xnb(nc.scalar, 1, lo, hi)
            _xnb(nc.gpsimd, 0, N // 2, N)
            _xnb(nc.gpsimd, 1, N // 2, N)

            # combined projection bias (off the critical path — only the
            # residual xnr needs it): bpc = bp + bv @ Wp; shift_r per cb.
            bpc_pp = consts.tile([P, 2], F32, tag="bpc")
            for cob in range(2):
                ps_bb = psA.tile([P, 1], F32, tag="bb", name=f"bb{cob}")
                for kb2 in range(2):
                    nc.tensor.matmul(
                        ps_bb,
                        lhsT=w_sbs["p"][:, kb2, cob * P:(cob + 1) * P],
                        rhs=bv_bf[:, kb2:kb2 + 1],
                        start=(kb2 == 0), stop=(kb2 == 1))
                nc.vector.tensor_tensor(out=bpc_pp[:, cob:cob + 1],
                                        in0=ps_bb,
                                        in1=bp_pp[:, cob:cob + 1],
                                        op=mybir.AluOpType.add)
            for cb in range(2):
                nc.vector.tensor_tensor(out=all6[:, 4 + cb:5 + cb],
                                        in0=all6[:, 2 + cb:3 + cb],
                                        in1=bpc_pp[:, cb:cb + 1],
                                        op=mybir.AluOpType.add)

            # ---- phase B: k^T fully + q^T for qt0 (fp8 DoubleRow) ----
            # Only what gates the first attention slots runs up front; the
            # rest of q^T and all of v are interleaved into qt0's slots.
            qT = big.tile([P, 2, NQ], FP8, tag="qT")
            kT = big.tile([P, 2, N], FP8, tag="kT")
            for nt in range(N // FD):
                for cob in range(2):
                    ps = psB.tile([P, FD], F32, tag="qk", name="ps_k")
                    nc.tensor.matmul(
                        ps, lhsT=w_sbs["k"][:, :, cob * P:(cob + 1) * P],
                        rhs=xnb[:, :, nt * FD:(nt + 1) * FD],
                        start=True, stop=True, perf_mode=DR)
                    nc.vector.tensor_scalar_add(
                        kT[:, cob, nt * FD:(nt + 1) * FD], ps,
                        bk_pp[:, cob:cob + 1])
            for cob in range(2):
                ps = psB.tile([P, FD], F32, tag="qk", name="ps_q0")
                nc.tensor.matmul(
                    ps, lhsT=w_sbs["q"][:, :, cob * P:(cob + 1) * P],
                    rhs=xnb[:, :, 0:FD],
                    start=True, stop=True, perf_mode=DR)
                nc.vector.tensor_scalar_add(
                    qT[:, cob, 0:FD], ps, bq_pp[:, cob:cob + 1])

        VROW = 272  # C+1 rounded up to a 16-byte interleave stride
        v_sb = big.tile([P, N // P, VROW], FP8, tag="v")
        nc.vector.memset(v_sb[:, :, C:C + 1], 1.0)

        # ---- phase C: attention, software-pipelined across q-tiles ----

        # ---- phase C: attention, software-pipelined across q-tiles ----
        # PSUM budget (8 banks): psS 2x[P,2,FD] = 4 (scores/exp staging; its
        # rotation also absorbs boundary transposes + projection tiles),
        # psO 2 (pass1 accumulators qb0/1), psP 2 (pass2 accumulators qb2/3).
        with tc.tile_pool(name="psS", bufs=2, space="PSUM") as psS, \
             tc.tile_pool(name="psO", bufs=1, space="PSUM") as psO, \
             tc.tile_pool(name="psP", bufs=1, space="PSUM") as psP, \
             tc.tile_pool(name="epool", bufs=2) as epool, \
             tc.tile_pool(name="apool", bufs=2) as apool, \
             tc.tile_pool(name="fpool", bufs=2) as fpool:

            NG = NKB // 2  # 16 groups of 2 key blocks per q-tile
            eT = {}        # qt -> eT_all tile
            ps1 = {}       # qt -> [ps_on qb0, ps_on qb1]
            ps2 = {}       # qt -> [ps_on qb2, ps_on qb3]
            aT = {}        # qt -> aT tile

            def extract(qt, qb, ps_acc, pool, tag):
                # per-row normalize (colsum is a per-partition scalar), then
                # transpose attn rows back to [C, q] for the projection.
                # Transposes land in the accumulator's own (just-freed) bank
                # so the score pipeline's psS rotation stays clean.
                rcp = small.tile([P, 1], F32, tag="rcp", name=f"rcp{qt}_{qb}")
                nc.vector.reciprocal(out=rcp, in_=ps_acc[:, C:C + 1])
                a_nat = work.tile([P, C], BF16, tag="a_nat")
                nc.vector.tensor_scalar_mul(a_nat, ps_acc[:, 0:C], rcp)
                for cb in range(2):
                    ps_at = pool.tile([P, P], BF16, tag=tag,
                                      name=f"ps_at{qt}_{qb}_{cb}")
                    nc.tensor.transpose(ps_at, a_nat[:, cb * P:(cb + 1) * P],
                                        ident_bf)
                    nc.vector.tensor_copy(
                        out=aT[qt][:, cb, qb * P:(qb + 1) * P], in_=ps_at)

            def proj_and_out(qt):
                qs = qt * FD
                for cob in range(2):
                    ps_f = psP.tile([P, FD], F32, tag=f"on{2 + cob}",
                                    name=f"ps_f{qt}_{cob}")
                    for kb2 in range(2):
                        nc.tensor.matmul(
                            ps_f,
                            lhsT=w_sbs["p"][:, kb2, cob * P:(cob + 1) * P],
                            rhs=aT[qt][:, kb2, :],
                            start=(kb2 == 0), stop=(kb2 == 1))
                    fo = fpool.tile([P, FD], F32, tag=f"fo{cob}")
                    nc.vector.tensor_add(out=fo, in0=ps_f,
                                         in1=xnr[:, cob, qs:qs + FD])
                    # gpsimd queue for half the stores: it is idle during
                    # attention, and a dispatch on the scalar queue would
                    # stall the exp stream
                    oq = nc.sync if cob == 0 else nc.gpsimd
                    oq.dma_start(out=out[cob, :, qs:qs + FD], in_=fo)

            def vproj(rb):
                psv = psS.tile([P, C], F32, tag="s", name=f"psv{rb}")
                nc.tensor.matmul(psv,
                                 lhsT=xnb[:, :, rb * P:(rb + 1) * P],
                                 rhs=w_sbs["v"],
                                 start=True, stop=True, perf_mode=DR)
                nc.vector.tensor_copy(out=v_sb[:, rb, 0:C], in_=psv)

            def qproj(nt):
                for cob in range(2):
                    ps = psS.tile([P, FD], F32, tag="s", name=f"ps_q{nt}")
                    nc.tensor.matmul(
                        ps, lhsT=w_sbs["q"][:, :, cob * P:(cob + 1) * P],
                        rhs=xnb[:, :, nt * FD:(nt + 1) * FD],
                        start=True, stop=True, perf_mode=DR)
                    nc.vector.tensor_scalar_add(
                        qT[:, cob, nt * FD:(nt + 1) * FD], ps,
                        bq_pp[:, cob:cob + 1])

            def pass1_mm(qt, gp):
                for qb in range(2):
                    nc.tensor.matmul(
                        ps1[qt][qb],
                        lhsT=eT[qt][:, 2 * gp:2 * gp + 2,
                                    qb * P:(qb + 1) * P],
                        rhs=v_sb[:, 2 * gp:2 * gp + 2, 0:C + 1],
                        start=(gp == 0), stop=(gp == NG - 1),
                        skip_group_check=True, perf_mode=DR)

            def pass2_mm(qt, gp):
                for h in range(2):
                    qb = 2 + h
                    nc.tensor.matmul(
                        ps2[qt][h],
                        lhsT=eT[qt][:, 2 * gp:2 * gp + 2,
                                    qb * P:(qb + 1) * P],
                        rhs=v_sb[:, 2 * gp:2 * gp + 2, 0:C + 1],
                        start=(gp == 0), stop=(gp == NG - 1),
                        skip_group_check=True, perf_mode=DR)

            def scores_mm(qt, g):
                qs = qt * FD
                ps_s2 = psS.tile([P, 2, FD], F32, tag="s",
                                 name=f"s{qt}_{g}")
                for j in range(2):
                    kb = 2 * g + j
                    nc.tensor.matmul(
                        ps_s2[:, j, :],
                        lhsT=kT[:, :, kb * P:(kb + 1) * P],
                        rhs=qT[:, :, qs:qs + FD],
                        start=True, stop=True, perf_mode=DR)
                nc.scalar.activation(
                    out=eT[qt][:, 2 * g:2 * g + 2, :],
                    in_=ps_s2, func=AF.Exp, scale=SCALE)

            # Pipeline across q-tiles. In iteration qt:
            #   - scores/exp of qt (slots), pass1(qt) trailing by 2 slots,
            #     pass2(qt-1) trailing by 2 slots
            #   - finalization of qt-2 (extract qb2/3 + proj + out) tucked
            #     between slots 1 and 2 so the ACT exp stream never waits
            #     behind boundary work on the PE queue
            #   - extract of qt's qb0/1 at iteration end (frees psO for the
            #     next iteration's pass1 allocations)
            for qt in range(NQT + 1):
                cur = qt < NQT
                prev = qt - 1 if qt > 0 else None
                if cur:
                    eT[qt] = epool.tile([P, NKB, FD], FP8, tag="eT",
                                        name=f"eT{qt}")
                    ps1[qt] = [psO.tile([P, C + 1], F32, tag=f"on{h}",
                                        name=f"ps1_{qt}_{h}")
                               for h in range(2)]
                    scores_mm(qt, 0)
                    scores_mm(qt, 1)
                if qt == 0:
                    for rb in range(6):
                        vproj(rb)
                # finalize qt-2 (its pass2 finished during iteration qt-1)
                if qt >= 2:
                    for h in range(2):
                        extract(qt - 2, 2 + h, ps2[qt - 2][h], psP,
                                f"on{2 + h}")
                    proj_and_out(qt - 2)
                if prev is not None:
                    ps2[prev] = [psP.tile([P, C + 1], F32, tag=f"on{2 + h}",
                                          name=f"ps2_{prev}_{h}")
                                 for h in range(2)]
                for g in range(2, NG):
                    if qt == 0:
                        if g < 15:
                            vproj(2 * g + 2)
                            vproj(2 * g + 3)
                        if g in (5, 9, 13):
                            qproj((g - 1) // 4)
                    if cur:
                        pass1_mm(qt, g - 2)
                    if prev is not None:
                        pass2_mm(prev, g - 2)
                    if cur:
                        scores_mm(qt, g)
                # trailing groups
                if cur:
                    pass1_mm(qt, NG - 2)
                if prev is not None:
                    pass2_mm(prev, NG - 2)
                if cur:
                    pass1_mm(qt, NG - 1)
                if prev is not None:
                    pass2_mm(prev, NG - 1)
                if cur:
                    aT[qt] = apool.tile([P, 2, FD], BF16, tag="aT",
                                        name=f"aT{qt}")
                    for qb in range(2):
                        extract(qt, qb, ps1[qt][qb], psO, f"on{qb}")
                if qt == 0:
                    # residual xn + bpc: first needed by proj(qt0) at the
                    # top of iteration 2, issued here so the preamble's
                    # attention-gating DVE copies all run first.
                    for cb in range(2):
                        for hf in range(2):
                            sl = slice(hf * (N // 2), (hf + 1) * (N // 2))
                            nc.vector.tensor_scalar(
                                out=xnr[:, cb, sl], in0=xT[:, cb, sl],
                                scalar1=all6[:, cb:cb + 1],
                                scalar2=all6[:, 4 + cb:5 + cb],
                                op0=mybir.AluOpType.mult,
                                op1=mybir.AluOpType.add)
            # final: qt = NQT-1's pass2 results
            for h in range(2):
                extract(NQT - 1, 2 + h, ps2[NQT - 1][h], psP, f"on{2 + h}")
            proj_and_out(NQT - 1)


_NC_CACHE = None


def _get_nc():
    global _NC_CACHE
    if _NC_CACHE is None:
        _NC_CACHE = build_nc()
    return _NC_CACHE


_FN_CACHE = None


def _get_fn():
    """Compile once; return fn. fn takes the concatenated blob
    [8*BLOB_SIZE] plus a zero output buffer and runs all 8 cores."""
    global _FN_CACHE
    if _FN_CACHE is None:
        import jax
        from jax.experimental.shard_map import shard_map
        from jax.sharding import Mesh, PartitionSpec
        from concourse.bass2jax import (
            _bass_exec_p,
            install_neuronx_cc_hook,
            partition_id_tensor,
        )

        install_neuronx_cc_hook()
        nc = _get_nc()
        partition_name = (
            nc.partition_id_tensor.name if nc.partition_id_tensor else None
        )
        in_names, out_names, out_avals = [], [], []
        for alloc in nc.m.functions[0].allocations:
            if not isinstance(alloc, mybir.MemoryLocationSet):
                continue
            name = alloc.memorylocations[0].name
            if alloc.kind == "ExternalInput":
                if name != partition_name:
                    in_names.append(name)
            elif alloc.kind == "ExternalOutput":
                out_names.append(name)
                out_avals.append(
                    jax.core.ShapedArray(tuple(alloc.tensor_shape),
                                         mybir.dt.np(alloc.dtype)))
        assert in_names == ["blob"] and out_names == ["out"]
        all_in = in_names + out_names + (
            [partition_name] if partition_name else [])

        def _jbody(*args):
            ops = list(args)
            if partition_name:
                ops.append(partition_id_tensor())
            return tuple(_bass_exec_p.bind(
                *ops, out_avals=tuple(out_avals), in_names=tuple(all_in),
                out_names=tuple(out_names), lowering_input_output_aliases=(),
                sim_require_finite=True, sim_require_nnan=True, nc=nc))

        mesh = Mesh(np.asarray(jax.devices()[:8]), ("core",))
        fn = jax.jit(
            shard_map(_jbody, mesh=mesh,
                      in_specs=(PartitionSpec("core"),) * 2,
                      out_specs=(PartitionSpec("core"),), check_rep=False),
            keep_unused=True)
        _FN_CACHE = fn
    return _FN_CACHE


def _egrp_const() -> np.ndarray:
    """[P, 2G] one-hot: egrp[p, cb*G+g] = 1 iff channel cb*P+p is in group g."""
    e = np.zeros((P, 2 * G), dtype=np.float32)
    for cb in range(2):
        for p in range(P):
            e[p, cb * G + (cb * P + p) // CG] = 1.0
    return e


def _egrpt_const() -> np.ndarray:
    """[G, C] one-hot transpose: egrpt[g, c] = 1 iff group(c) == g."""
    e = np.zeros((G, C), dtype=np.float32)
    for c in range(C):
        e[c // CG, c] = 1.0
    return e


def _to_bf16_bits(a: np.ndarray) -> np.ndarray:
    """Round f32 -> bf16 (round-to-nearest-even) and return uint16 bits."""
    u = np.ascontiguousarray(a, dtype=np.float32).view(np.uint32)
    rounded = (u + 0x7FFF + ((u >> 16) & 1)) >> 16
    return rounded.astype(np.uint16)


def make_in_maps(inputs: dict) -> list[dict]:
    x = np.ascontiguousarray(np.asarray(inputs["x"], dtype=np.float32))
    x_flat = x.reshape(B, N, C)
    shared = np.concatenate([
        np.asarray(inputs["Wq"], np.float32).ravel(),
        np.asarray(inputs["Wk"], np.float32).ravel(),
        np.asarray(inputs["Wv"], np.float32).ravel(),
        np.asarray(inputs["Wp"], np.float32).ravel(),
        np.asarray(inputs["bq"], np.float32).ravel(),
        np.asarray(inputs["bk"], np.float32).ravel(),
        np.asarray(inputs["bv"], np.float32).ravel(),
        np.asarray(inputs["bp"], np.float32).ravel(),
        np.asarray(inputs["gamma"], np.float32).ravel(),
        np.asarray(inputs["beta"], np.float32).ravel(),
        np.eye(P, dtype=np.float32).ravel(),
        _egrp_const().ravel(),
        _egrpt_const().ravel(),
    ])
    in_maps = []
    for core in range(8):
        b, h = core // 2, core % 2
        if h == 0:
            xp = x_flat[b]
        else:
            xp = np.concatenate([x_flat[b, NQ:], x_flat[b, :NQ]], axis=0)
        # channel-major blob layout: x[cb, p, n] = xp[n, cb*128+p],
        # rounded to bf16 and packed into the first N*C/2 f32 words
        xp_cm = np.ascontiguousarray(xp.T.reshape(2, P, N))
        xp_bf = _to_bf16_bits(xp_cm)
        xp_packed = np.zeros(N * C, np.float32)
        xp_packed.view(np.uint16)[0:N * C] = xp_bf.ravel()
        in_maps.append({"blob": np.concatenate([xp_packed, shared])})
    return in_maps


def assemble(results: list[dict]) -> np.ndarray:
    y = np.empty((B, N, C), dtype=np.float32)
    for core in range(8):
        b, h = core // 2, core % 2
        o = results[core]["out"]  # [2, 128, NQ] channel-major
        y[b, h * NQ:(h + 1) * NQ] = o.reshape(C, NQ).T
    return y.reshape(B, H, W, C)


def kernel(**inputs) -> np.ndarray:
    fn = _get_fn()
    in_maps = make_in_maps(inputs)
    blob = np.concatenate([m["blob"] for m in in_maps])
    zeros = np.zeros((8 * 2, P, NQ), np.float32)
    (out,) = fn(blob, zeros)
    out = np.asarray(out).reshape(8, 2, P, NQ)
    return assemble([{"out": out[c]} for c in range(8)])


# revision 42
# speedup vs baseline: 1.0012x; 1.0012x over previous
"""Self-contained Trainium2 Bass kernel for nn_AttentionBlock (GroupNorm +
single-head attention + residual).

Reference computation (shapes hardcoded):
    x: [B=4, H=64, W=64, C=256] f32
    xn = GroupNorm(x, groups=8, eps=1e-3) * gamma + beta
    q/k/v = xn @ W{q,k,v} + b{q,k,v}           # per batch, N=H*W=4096 tokens
    attn = softmax(q @ k^T / sqrt(C))
    out  = xn + (attn @ v) @ Wp + bp

Sharding: 8 cores = (batch b, query-half h). Each core receives its batch's
full token sequence (channel-major / pre-transposed on the host) with rows
rotated so its 2048 query rows come first; it computes groupnorm + K/V for
all 4096 tokens and attention for its 2048 query rows. No collectives. The
host reassembles the 8 transposed [2,128,2048] outputs.

Key layout choices:
  - x arrives channel-major [cb, p, n] AND rounded to bf16 on the host (no
    on-device input transposes, half the input DMA bytes; the bf16
    rounding costs ~3e-3 relative error against a 2e-2 budget).
  - x loads as 6 sub-DMAs spread over the 3 DMA-capable queues
    (sync/scalar/gpsimd) so groupnorm stats chunk-chase the transfers.
  - q/k/v projections + scores + attn@v run in fp8 with DoubleRow (2x).
  - exp'd scores stay key-major (eT); attn@v uses the ones-column trick for
    softmax row sums; all biases are folded into existing copies/residual
    (bv rides the projection bias since softmax rows sum to one).
  - attention is software-pipelined across q-tiles: per 2-key-block slot
    the PE runs [pass1(qt,g-2), pass2(qt-1,g-2), scores(qt,g)] while the
    ACT engine exps the previous slot's scores; v and the remaining q
    projections are interleaved into qt0's slots; extract/projection
    boundary work lands in the accumulators' own just-freed PSUM banks so
    the score pipeline's PSUM rotation never blocks the exp stream.
  - output leaves the device transposed [cob, p, n]; host de-transposes.
"""

import numpy as np

import concourse.bass as bass
import concourse.tile as tile
from concourse import mybir
from concourse.tile import ScopedClock

# Problem shapes (hardcoded per contract)
B, H, W, C = 4, 64, 64, 256
N = H * W            # 4096 tokens per batch image
NQ = N // 2          # 2048 query rows per core
G = 8                # groupnorm groups
CG = C // G          # 32 channels per group
EPS = 1e-3
P = 128
FD = 512             # matmul moving-operand free dim (one PSUM bank of f32)
NKB = N // P         # 32 key blocks
NQT = NQ // FD       # 4 query tiles per core
SCALE = float(C) ** -0.5
F32 = mybir.dt.float32
BF16 = mybir.dt.bfloat16
FP8 = mybir.dt.float8e4
AF = mybir.ActivationFunctionType
DR = mybir.MatmulPerfMode.DoubleRow

# dev knob: repeat the whole body R times inside one NEFF (throughput probe)
REPS = 1
LOOPN = 1
VARIANT = "full"


def _drain_and_barrier_split(self, tick_clock, wait_clock):
    """Replacement for TileContext._drain_and_barrier.

    The walrus build in this container rejects sem waits on InstDrain (and
    >1 wait on a NOP), so carry the end-of-kernel waits on a chain of NOPs
    with one wait each, drain without sync, and use the sem-only (no-Drain)
    all-engine barrier around semaphore cleanup.
    """
    nc = self.nc
    carrier = nc.sync.nop(nofuse=True)
    wait_clock.add_sem_waits(
        carrier.ins, ScopedClock({None: tick_clock.global_clock})
    )
    si = carrier.ins.sync_info
    waits = list(si.on_wait) if si is not None and si.on_wait else []
    if len(waits) > 1:
        carrier.ins.sync_info = mybir.SyncInfo(
            on_wait=waits[:1], on_update=list(si.on_update or [])
        )
        for w in waits[1:]:
            extra = nc.sync.nop(nofuse=True)
            extra.ins.sync_info = mybir.SyncInfo(on_wait=[w], on_update=[])
    nc.sync.drain()
    nc.all_engine_barrier(sem_only=True)
    assert self.sems is not None
    popped = nc._tile_sem_poison_stack.pop()
    assert popped is self._sem_poison
    nc.clear_and_free_semaphores(list(self.sems.allocated().values()))
    nc.all_engine_barrier(sem_only=True)


tile.TileContext._drain_and_barrier = _drain_and_barrier_split

_wsplit_ctr = 0


def _split_multi_waits(nc: bass.Bass):
    """Walrus in this container supports at most one sync wait per
    instruction (and none on Drain). Hoist excess waits onto NoOps placed
    just before the instruction on the same engine — sequencers process
    instructions in order, so blocking on the NoOp is equivalent."""
    global _wsplit_ctr
    for f in nc.m.functions:
        for bb in f.blocks:
            new_insts = []
            for ins in bb.instructions:
                si = getattr(ins, "sync_info", None)
                waits = list(si.on_wait) if si is not None and si.on_wait else []
                limit = 0 if ins.opcode == "Drain" else 1
                if len(waits) > limit:
                    keep = waits[len(waits) - limit:] if limit else []
                    hoist = waits[: len(waits) - limit]
                    for w in hoist:
                        _wsplit_ctr += 1
                        nop = mybir.InstNoOp(
                            name=f"I-wsplit-{_wsplit_ctr}",
                            engine=ins.engine,
                            sync_info=mybir.SyncInfo(on_wait=[w], on_update=[]),
                        )
                        new_insts.append(nop)
                    ins.sync_info = mybir.SyncInfo(
                        on_wait=keep, on_update=list(si.on_update or [])
                    )
                new_insts.append(ins)
            bb.instructions[:] = new_insts


# ---- single-blob input packing (one input param + one output param:
# each extra parameter costs ~2 ms/execution in this PJRT path) ----
_SEGS = [
    ("x", N * C),        # channel-major: [cb, p, n]
    ("wq", C * C), ("wk", C * C), ("wv", C * C), ("wp", C * C),
    ("bq", C), ("bk", C), ("bv", C), ("bp", C),
    ("gamma", C), ("beta", C),
    ("ident", P * P),
    ("egrp", P * 2 * G),
    ("egrpt", G * C),
]
_OFF = {}
_total = 0
for _nm, _sz in _SEGS:
    _OFF[_nm] = _total
    _total += _sz
BLOB_SIZE = _total


def build_nc(split_waits: bool = True) -> bass.Bass:
    nc = bass.Bass(enable_partition_id=False)
    blob = nc.declare_dram_parameter("blob", [BLOB_SIZE], F32, isOutput=False)[:]

    def seg(name, size):
        return blob[_OFF[name]:_OFF[name] + size]

    x = seg("x", N * C).bitcast(BF16)[0:N * C].rearrange(
        "(cb p n) -> p cb n", cb=2, p=P, n=N)
    wq = seg("wq", C * C).rearrange("(ci co) -> ci co", co=C)
    wk = seg("wk", C * C).rearrange("(ci co) -> ci co", co=C)
    wv = seg("wv", C * C).rearrange("(ci co) -> ci co", co=C)
    wp = seg("wp", C * C).rearrange("(ci co) -> ci co", co=C)
    bq = seg("bq", C)
    b6seg = seg("bq", 6 * C)
    bk = seg("bk", C)
    bv = seg("bv", C)
    bp = seg("bp", C)
    gamma = seg("gamma", C)
    beta = seg("beta", C)
    ident = seg("ident", P * P).rearrange("(a b) -> a b", b=P)
    egrp = seg("egrp", P * 2 * G).rearrange("(a b) -> a b", b=2 * G)
    egrpt = seg("egrpt", G * C).rearrange("(a b) -> a b", b=C)
    # transposed output: out[cob, p, n] = channel cob*128+p of query row n
    out = nc.declare_dram_parameter("out", [2, P, NQ], F32, isOutput=True)[:]

    with tile.TileContext(nc) as tc:
        if LOOPN > 1:
            with tc.For_i(0, LOOPN, 1):
                _body(nc, tc, x, wq, wk, wv, wp, b6seg, ident, egrp,
                      egrpt, out)
        else:
            for _rep in range(REPS):
                _body(nc, tc, x, wq, wk, wv, wp, b6seg, ident, egrp,
                      egrpt, out)
    if split_waits:
        _split_multi_waits(nc)
    return nc


def _body(nc, tc, x, wq, wk, wv, wp, b6seg, ident, egrp, egrpt, out):
    from contextlib import ExitStack
    ctx = ExitStack()
    with ctx:
        consts = ctx.enter_context(tc.tile_pool(name="consts", bufs=1))
        big = ctx.enter_context(tc.tile_pool(name="big", bufs=1))
        work = ctx.enter_context(tc.tile_pool(name="work", bufs=3))
        small = ctx.enter_context(tc.tile_pool(name="small", bufs=2))

        # ---- x^T first: 3 big chunks, one per DMA-capable queue (a single
        # queue tops out well below the core's aggregate HBM bandwidth, and
        # each dma_start costs ~1us of dispatch on its host queue) ----
        xT = big.tile([P, 2, N], BF16, tag="xT")      # raw input, ch-major
        NCH = 4
        nch = N // NCH
        _X6 = ((nc.sync, 0, 1024), (nc.scalar, 1536, 2560),
               (nc.gpsimd, 3072, 3584), (nc.sync, 1024, 1536),
               (nc.scalar, 2560, 3072), (nc.gpsimd, 3584, 4096))
        for eng, lo, hi in _X6:
            eng.dma_start(out=xT[:, :, lo:hi], in_=x[:, :, lo:hi])

        # small constants: bq/bk/bv/bp/gamma/beta are consecutive [2,P]
        # blob segments — one batched DMA instead of six dispatches
        b6 = consts.tile([P, 12], F32, tag="b6")
        nc.gpsimd.dma_start(
            out=b6, in_=b6seg.rearrange("(k b p) -> p (k b)", b=2, p=P))
        bq_pp = b6[:, 0:2]
        bk_pp = b6[:, 2:4]
        bv_pp = b6[:, 4:6]
        bp_pp = b6[:, 6:8]
        gamma_pp = b6[:, 8:10]
        beta_pp = b6[:, 10:12]
        egrp_sb = consts.tile([P, 2 * G], F32, tag="egrp")
        nc.gpsimd.dma_start(out=egrp_sb, in_=egrp)
        egrpt_sb = consts.tile([G, C], F32, tag="egrpt")
        nc.gpsimd.dma_start(out=egrpt_sb, in_=egrpt)
        ident_f = consts.tile([P, P], F32, tag="ident_f")
        nc.sync.dma_start(out=ident_f, in_=ident)

        # weights: stage f32 (scalar queue, behind its x chunk); the fp8 /
        # bf16 casts are issued later so the DVE queue serves stats first
        w_stage = {}
        for name, wh in (("q", wq), ("k", wk), ("v", wv), ("p", wp)):
            tf = work.tile([P, 2, C], F32, tag=f"wstage_{name}",
                           name=f"wstage_{name}")
            nc.scalar.dma_start(
                out=tf, in_=wh.rearrange("(kb p) co -> p kb co", p=P))
            w_stage[name] = tf

        # ---- phase A: stats, normalize ----
        xnr = big.tile([P, 2, N], F32, tag="xnr")     # xn + bpc (residual)
        xnb = big.tile([P, 2, N], FP8, tag="xnb")     # xn fp8 for projections

        with tc.tile_pool(name="psA", bufs=1, space="PSUM") as psA, \
             tc.tile_pool(name="psB", bufs=2, space="PSUM") as psB:
            # per-channel mean/var via bn_stats (channels on partitions),
            # chunk-chasing the (3-queue-parallel) x DMA.
            ps_g = psA.tile([G, 2], F32, tag="gstats")
            fmax = nc.vector.BN_STATS_FMAX
            nsub = N // fmax
            stats = [work.tile([P, nsub, nc.vector.BN_STATS_DIM], F32,
                               tag=f"bnstats{cb}", name=f"bnstats{cb}")
                     for cb in range(2)]
            _ARRIVE = (0, 1, 3, 4, 6, 2, 5, 7)  # slice order per _X6
            for si in _ARRIVE:
                for cb in range(2):
                    nc.vector.bn_stats(
                        out=stats[cb][:, si, :],
                        in_=xT[:, cb, si * fmax:(si + 1) * fmax])
            for cb in range(2):
                mv = work.tile([P, nc.vector.BN_AGGR_DIM], F32, tag="bnmv")
                nc.vector.bn_aggr(out=mv, in_=stats[cb])
                # pack (mean_c, E[x^2]_c = var_c + mean_c^2)
                pk = work.tile([P, 2], F32, tag="pk")
                nc.vector.tensor_copy(out=pk[:, 0:1], in_=mv[:, 0:1])
                msq = work.tile([P, 1], F32, tag="msq")
                nc.vector.tensor_mul(out=msq, in0=mv[:, 0:1], in1=mv[:, 0:1])
                nc.vector.tensor_add(out=pk[:, 1:2], in0=mv[:, 1:2], in1=msq)
                nc.tensor.matmul(ps_g, lhsT=egrp_sb[:, cb * G:(cb + 1) * G],
                                 rhs=pk, start=(cb == 0), stop=(cb == 1),
                                 skip_group_check=True)

            # finalize on G partitions: mean_g, rstd_g
            gsb = small.tile([G, 2], F32, tag="gsb")
            nc.vector.tensor_scalar_mul(gsb, ps_g, 1.0 / CG)
            gmean = gsb[:, 0:1]
            gex2 = gsb[:, 1:2]
            gmsq = small.tile([G, 1], F32, tag="gmsq")
            nc.vector.tensor_mul(out=gmsq, in0=gmean, in1=gmean)
            gvar = small.tile([G, 1], F32, tag="gvar")
            nc.vector.tensor_tensor(out=gvar, in0=gex2, in1=gmsq,
                                    op=mybir.AluOpType.subtract)
            eps_sb = small.tile([G, 1], F32, tag="eps")
            nc.vector.memset(eps_sb, EPS)
            gstd = small.tile([G, 1], F32, tag="gstd")
            nc.scalar.activation(out=gstd, in_=gvar, func=AF.Sqrt, bias=eps_sb)
            gpack = small.tile([G, 2], F32, tag="gpack")
            nc.vector.tensor_copy(out=gpack[:, 0:1], in_=gmean)
            nc.vector.reciprocal(out=gpack[:, 1:2], in_=gstd)

            # deferred casts (weights fp8/bf16, identity bf16, bv bf16) —
            # issued after stats so the DVE queue serves stats first
            w_sbs = {}
            for name, dt in (("q", FP8), ("k", FP8), ("v", FP8),
                             ("p", BF16)):
                t = consts.tile([P, 2, C], dt, tag=f"w{name}",
                                name=f"w{name}")
                nc.vector.tensor_copy(out=t, in_=w_stage[name])
                w_sbs[name] = t
            ident_bf = consts.tile([P, P], BF16, tag="ident_bf")
            nc.vector.tensor_copy(out=ident_bf, in_=ident_f)
            bv_bf = consts.tile([P, 2], BF16, tag="bv_bf")
            nc.vector.tensor_copy(out=bv_bf, in_=bv_pp)

            # broadcast group stats back to channel partitions; build
            # all6[:, cb]   = scale_c  = rstd * gamma
            # all6[:, 2+cb] = shift_c  = beta - mean*scale
            # all6[:, 4+cb] = shift_c + bpc_c   (residual version)
            all6 = consts.tile([P, 6], F32, tag="all6")
            for cb in range(2):
                ps_bc = psA.tile([P, 2], F32, tag="bc", name=f"bc{cb}")
                nc.tensor.matmul(ps_bc, lhsT=egrpt_sb[:, cb * P:(cb + 1) * P],
                                 rhs=gpack, start=True, stop=True)
                mr = small.tile([P, 2], F32, tag="mr")
                nc.vector.tensor_copy(out=mr, in_=ps_bc)
                nc.vector.tensor_mul(out=all6[:, cb:cb + 1], in0=mr[:, 1:2],
                                     in1=gamma_pp[:, cb:cb + 1])
                ms = small.tile([P, 1], F32, tag="ms")
                nc.vector.tensor_mul(out=ms, in0=mr[:, 0:1],
                                     in1=all6[:, cb:cb + 1])
                nc.vector.tensor_tensor(out=all6[:, 2 + cb:3 + cb],
                                        in0=beta_pp[:, cb:cb + 1], in1=ms,
                                        op=mybir.AluOpType.subtract)

            # normalize: xnb (fp8, for projections). The n-range that gates
            # the first score groups (h0) runs on gpsimd (cb0) + scalar
            # (cb1) in quarter chunks; the later h1 range follows on
            # gpsimd. The vector queue is left free for the weight casts
            # and k^T copies that also gate the first scores.
            def _xnb(eng, cb, lo, hi):
                if eng is nc.scalar:
                    nc.scalar.activation(
                        out=xnb[:, cb, lo:hi], in_=xT[:, cb, lo:hi],
                        func=AF.Identity, scale=all6[:, cb:cb + 1],
                        bias=all6[:, 2 + cb:3 + cb])
                else:
                    eng.tensor_scalar(
                        out=xnb[:, cb, lo:hi], in0=xT[:, cb, lo:hi],
                        scalar1=all6[:, cb:cb + 1],
                        scalar2=all6[:, 2 + cb:3 + cb],
                        op0=mybir.AluOpType.mult, op1=mybir.AluOpType.add)

            for qtr in range(2):
                lo, hi = qtr * (N // 4), (qtr + 1) * (N // 4)
                _xnb(nc.gpsimd, 0, lo, hi)
                _xnb(nc.scalar, 1, lo, hi)
            _xnb(nc.gpsimd, 0, N // 2, N)
            _xnb(nc.gpsimd, 1, N // 2, N)

            # combined projection bias (off the critical path — only the
            # residual xnr needs it): bpc = bp + bv @ Wp; shift_r per cb.
            bpc_pp = consts.tile([P, 2], F32, tag="bpc")
            for cob in range(2):
                ps_bb = psA.tile([P, 1], F32, tag="bb", name=f"bb{cob}")
                for kb2 in range(2):
                    nc.tensor.matmul(
                        ps_bb,
                        lhsT=w_sbs["p"][:, kb2, cob * P:(cob + 1) * P],
                        rhs=bv_bf[:, kb2:kb2 + 1],
                        start=(kb2 == 0), stop=(kb2 == 1))
                nc.vector.tensor_tensor(out=bpc_pp[:, cob:cob + 1],
                                        in0=ps_bb,
                                        in1=bp_pp[:, cob:cob + 1],
                                        op=mybir.AluOpType.add)
            for cb in range(2):
                nc.vector.tensor_tensor(out=all6[:, 4 + cb:5 + cb],
                                        in0=all6[:, 2 + cb:3 + cb],
                                        in1=bpc_pp[:, cb:cb + 1],
                                        op=mybir.AluOpType.add)

            # ---- phase B: k^T fully + q^T for qt0 (fp8 DoubleRow) ----
            # Only what gates the first attention slots runs up front; the
            # rest of q^T and all of v are interleaved into qt0's slots.
            qT = big.tile([P, 2, NQ], FP8, tag="qT")
            kT = big.tile([P, 2, N], FP8, tag="kT")
            for nt in range(N // FD):
                for cob in range(2):
                    ps = psB.tile([P, FD], F32, tag="qk", name="ps_k")
                    nc.tensor.matmul(
                        ps, lhsT=w_sbs["k"][:, :, cob * P:(cob + 1) * P],
                        rhs=xnb[:, :, nt * FD:(nt + 1) * FD],
                        start=True, stop=True, perf_mode=DR)
                    nc.vector.tensor_scalar_add(
                        kT[:, cob, nt * FD:(nt + 1) * FD], ps,
                        bk_pp[:, cob:cob + 1])
            for cob in range(2):
                ps = psB.tile([P, FD], F32, tag="qk", name="ps_q0")
                nc.tensor.matmul(
                    ps, lhsT=w_sbs["q"][:, :, cob * P:(cob + 1) * P],
                    rhs=xnb[:, :, 0:FD],
                    start=True, stop=True, perf_mode=DR)
                nc.vector.tensor_scalar_add(
                    qT[:, cob, 0:FD], ps, bq_pp[:, cob:cob + 1])

        VROW = 272  # C+1 rounded up to a 16-byte interleave stride
        v_sb = big.tile([P, N // P, VROW], FP8, tag="v")
        nc.vector.memset(v_sb[:, :, C:C + 1], 1.0)

        # ---- phase C: attention, software-pipelined across q-tiles ----

        # ---- phase C: attention, software-pipelined across q-tiles ----
        # PSUM budget (8 banks): psS 2x[P,2,FD] = 4 (scores/exp staging; its
        # rotation also absorbs boundary transposes + projection tiles),
        # psO 2 (pass1 accumulators qb0/1), psP 2 (pass2 accumulators qb2/3).
        with tc.tile_pool(name="psS", bufs=2, space="PSUM") as psS, \
             tc.tile_pool(name="psO", bufs=1, space="PSUM") as psO, \
             tc.tile_pool(name="psP", bufs=1, space="PSUM") as psP, \
             tc.tile_pool(name="epool", bufs=2) as epool, \
             tc.tile_pool(name="apool", bufs=2) as apool, \
             tc.tile_pool(name="fpool", bufs=2) as fpool:

            NG = NKB // 2  # 16 groups of 2 key blocks per q-tile
            eT = {}        # qt -> eT_all tile
            ps1 = {}       # qt -> [ps_on qb0, ps_on qb1]
            ps2 = {}       # qt -> [ps_on qb2, ps_on qb3]
            aT = {}        # qt -> aT tile

            def extract(qt, qb, ps_acc, pool, tag):
                # per-row normalize (colsum is a per-partition scalar), then
                # transpose attn rows back to [C, q] for the projection.
                # Transposes land in the accumulator's own (just-freed) bank
                # so the score pipeline's psS rotation stays clean.
                rcp = small.tile([P, 1], F32, tag="rcp", name=f"rcp{qt}_{qb}")
                nc.vector.reciprocal(out=rcp, in_=ps_acc[:, C:C + 1])
                a_nat = work.tile([P, C], BF16, tag="a_nat")
                nc.vector.tensor_scalar_mul(a_nat, ps_acc[:, 0:C], rcp)
                for cb in range(2):
                    ps_at = pool.tile([P, P], BF16, tag=tag,
                                      name=f"ps_at{qt}_{qb}_{cb}")
                    nc.tensor.transpose(ps_at, a_nat[:, cb * P:(cb + 1) * P],
                                        ident_bf)
                    nc.vector.tensor_copy(
                        out=aT[qt][:, cb, qb * P:(qb + 1) * P], in_=ps_at)

            def proj_and_out(qt):
                qs = qt * FD
                for cob in range(2):
                    ps_f = psP.tile([P, FD], F32, tag=f"on{2 + cob}",
                                    name=f"ps_f{qt}_{cob}")
                    for kb2 in range(2):
                        nc.tensor.matmul(
                            ps_f,
                            lhsT=w_sbs["p"][:, kb2, cob * P:(cob + 1) * P],
                            rhs=aT[qt][:, kb2, :],
                            start=(kb2 == 0), stop=(kb2 == 1))
                    fo = fpool.tile([P, FD], F32, tag=f"fo{cob}")
                    nc.vector.tensor_add(out=fo, in0=ps_f,
                                         in1=xnr[:, cob, qs:qs + FD])
                    # gpsimd queue for half the stores: it is idle during
                    # attention, and a dispatch on the scalar queue would
                    # stall the exp stream
                    oq = nc.sync if cob == 0 else nc.gpsimd
                    oq.dma_start(out=out[cob, :, qs:qs + FD], in_=fo)

            def vproj(rb):
                psv = psS.tile([P, C], F32, tag="s", name=f"psv{rb}")
                nc.tensor.matmul(psv,
                                 lhsT=xnb[:, :, rb * P:(rb + 1) * P],
                                 rhs=w_sbs["v"],
                                 start=True, stop=True, perf_mode=DR)
                nc.vector.tensor_copy(out=v_sb[:, rb, 0:C], in_=psv)

            def qproj(nt):
                for cob in range(2):
                    ps = psS.tile([P, FD], F32, tag="s", name=f"ps_q{nt}")
                    nc.tensor.matmul(
                        ps, lhsT=w_sbs["q"][:, :, cob * P:(cob + 1) * P],
                        rhs=xnb[:, :, nt * FD:(nt + 1) * FD],
                        start=True, stop=True, perf_mode=DR)
                    nc.vector.tensor_scalar_add(
                        qT[:, cob, nt * FD:(nt + 1) * FD], ps,
                        bq_pp[:, cob:cob + 1])

            def pass1_mm(qt, gp):
                for qb in range(2):
                    nc.tensor.matmul(
                        ps1[qt][qb],
                        lhsT=eT[qt][:, 2 * gp:2 * gp + 2,
                                    qb * P:(qb + 1) * P],
                        rhs=v_sb[:, 2 * gp:2 * gp + 2, 0:C + 1],
                        start=(gp == 0), stop=(gp == NG - 1),
                        skip_group_check=True, perf_mode=DR)

            def pass2_mm(qt, gp):
                for h in range(2):
                    qb = 2 + h
                    nc.tensor.matmul(
                        ps2[qt][h],
                        lhsT=eT[qt][:, 2 * gp:2 * gp + 2,
                                    qb * P:(qb + 1) * P],
                        rhs=v_sb[:, 2 * gp:2 * gp + 2, 0:C + 1],
                        start=(gp == 0), stop=(gp == NG - 1),
                        skip_group_check=True, perf_mode=DR)

            def scores_mm(qt, g):
                qs = qt * FD
                ps_s2 = psS.tile([P, 2, FD], F32, tag="s",
                                 name=f"s{qt}_{g}")
                for j in range(2):
                    kb = 2 * g + j
                    nc.tensor.matmul(
                        ps_s2[:, j, :],
                        lhsT=kT[:, :, kb * P:(kb + 1) * P],
                        rhs=qT[:, :, qs:qs + FD],
                        start=True, stop=True, perf_mode=DR)
                nc.scalar.activation(
                    out=eT[qt][:, 2 * g:2 * g + 2, :],
                    in_=ps_s2, func=AF.Exp, scale=SCALE)

            # Pipeline across q-tiles. In iteration qt:
            #   - scores/exp of qt (slots), pass1(qt) trailing by 2 slots,
            #     pass2(qt-1) trailing by 2 slots
            #   - finalization of qt-2 (extract qb2/3 + proj + out) tucked
            #     between slots 1 and 2 so the ACT exp stream never waits
            #     behind boundary work on the PE queue
            #   - extract of qt's qb0/1 at iteration end (frees psO for the
            #     next iteration's pass1 allocations)
            for qt in range(NQT + 1):
                cur = qt < NQT
                prev = qt - 1 if qt > 0 else None
                if cur:
                    eT[qt] = epool.tile([P, NKB, FD], FP8, tag="eT",
                                        name=f"eT{qt}")
                    ps1[qt] = [psO.tile([P, C + 1], F32, tag=f"on{h}",
                                        name=f"ps1_{qt}_{h}")
                               for h in range(2)]
                    scores_mm(qt, 0)
                    scores_mm(qt, 1)
                if qt == 0:
                    for rb in range(6):
                        vproj(rb)
                # finalize qt-2 (its pass2 finished during iteration qt-1)
                if qt >= 2:
                    for h in range(2):
                        extract(qt - 2, 2 + h, ps2[qt - 2][h], psP,
                                f"on{2 + h}")
                    proj_and_out(qt - 2)
                if prev is not None:
                    ps2[prev] = [psP.tile([P, C + 1], F32, tag=f"on{2 + h}",
                                          name=f"ps2_{prev}_{h}")
                                 for h in range(2)]
                for g in range(2, NG):
                    if qt == 0:
                        if g < 15:
                            vproj(2 * g + 2)
                            vproj(2 * g + 3)
                        if g in (5, 9, 13):
                            qproj((g - 1) // 4)
                    if cur:
                        pass1_mm(qt, g - 2)
                    if prev is not None:
                        pass2_mm(prev, g - 2)
                    if cur:
                        scores_mm(qt, g)
                # trailing groups
                if cur:
                    pass1_mm(qt, NG - 2)
                if prev is not None:
                    pass2_mm(prev, NG - 2)
                if cur:
                    pass1_mm(qt, NG - 1)
                if prev is not None:
                    pass2_mm(prev, NG - 1)
                if cur:
                    aT[qt] = apool.tile([P, 2, FD], BF16, tag="aT",
                                        name=f"aT{qt}")
                    for qb in range(2):
                        extract(qt, qb, ps1[qt][qb], psO, f"on{qb}")
                if qt == 0:
                    # residual xn + bpc: first needed by proj(qt0) at the
                    # top of iteration 2, issued here so the preamble's
                    # attention-gating DVE copies all run first.
                    for cb in range(2):
                        for hf in range(2):
                            sl = slice(hf * (N // 2), (hf + 1) * (N // 2))
                            nc.vector.tensor_scalar(
                                out=xnr[:, cb, sl], in0=xT[:, cb, sl],
                                scalar1=all6[:, cb:cb + 1],
                                scalar2=all6[:, 4 + cb:5 + cb],
                                op0=mybir.AluOpType.mult,
                                op1=mybir.AluOpType.add)
            # final: qt = NQT-1's pass2 results
            for h in range(2):
                extract(NQT - 1, 2 + h, ps2[NQT - 1][h], psP, f"on{2 + h}")
            proj_and_out(NQT - 1)


_NC_CACHE = None


def _get_nc():
    global _NC_CACHE
    if _NC_CACHE is None:
        _NC_CACHE = build_nc()
    return _NC_CACHE


_FN_CACHE = None


def _get_fn():
    """Compile once; return fn. fn takes the concatenated blob
    [8*BLOB_SIZE] plus a zero output buffer and runs all 8 cores."""
    global _FN_CACHE
    if _FN_CACHE is None:
        import jax
        from jax.experimental.shard_map import shard_map
        from jax.sharding import Mesh, PartitionSpec
        from concourse.bass2jax import (
            _bass_exec_p,
            install_neuronx_cc_hook,
            partition_id_tensor,
        )

        install_neuronx_cc_hook()
        nc = _get_nc()
        partition_name = (
            nc.partition_id_tensor.name if nc.partition_id_tensor else None
        )
        in_names, out_names, out_avals = [], [], []
        for alloc in nc.m.functions[0].allocations:
            if not isinstance(alloc, mybir.MemoryLocationSet):
                continue
            name = alloc.memorylocations[0].name
            if alloc.kind == "ExternalInput":
                if name != partition_name:
                    in_names.append(name)
            elif alloc.kind == "ExternalOutput":
                out_names.append(name)
                out_avals.append(
                    jax.core.ShapedArray(tuple(alloc.tensor_shape),
                                         mybir.dt.np(alloc.dtype)))
        assert in_names == ["blob"] and out_names == ["out"]
        all_in = in_names + out_names + (
            [partition_name] if partition_name else [])

        def _jbody(*args):
            ops = list(args)
            if partition_name:
                ops.append(partition_id_tensor())
            return tuple(_bass_exec_p.bind(
                *ops, out_avals=tuple(out_avals), in_names=tuple(all_in),
                out_names=tuple(out_names), lowering_input_output_aliases=(),
                sim_require_finite=True, sim_require_nnan=True, nc=nc))

        mesh = Mesh(np.asarray(jax.devices()[:8]), ("core",))
        fn = jax.jit(
            shard_map(_jbody, mesh=mesh,
                      in_specs=(PartitionSpec("core"),) * 2,
                      out_specs=(PartitionSpec("core"),), check_rep=False),
            keep_unused=True)
        _FN_CACHE = fn
    return _FN_CACHE


def _egrp_const() -> np.ndarray:
    """[P, 2G] one-hot: egrp[p, cb*G+g] = 1 iff channel cb*P+p is in group g."""
    e = np.zeros((P, 2 * G), dtype=np.float32)
    for cb in range(2):
        for p in range(P):
            e[p, cb * G + (cb * P + p) // CG] = 1.0
    return e


def _egrpt_const() -> np.ndarray:
    """[G, C] one-hot transpose: egrpt[g, c] = 1 iff group(c) == g."""
    e = np.zeros((G, C), dtype=np.float32)
    for c in range(C):
        e[c // CG, c] = 1.0
    return e


def _to_bf16_bits(a: np.ndarray) -> np.ndarray:
    """Round f32 -> bf16 (round-to-nearest-even) and return uint16 bits."""
    u = np.ascontiguousarray(a, dtype=np.float32).view(np.uint32)
    rounded = (u + 0x7FFF + ((u >> 16) & 1)) >> 16
    return rounded.astype(np.uint16)


def make_in_maps(inputs: dict) -> list[dict]:
    x = np.ascontiguousarray(np.asarray(inputs["x"], dtype=np.float32))
    x_flat = x.reshape(B, N, C)
    shared = np.concatenate([
        np.asarray(inputs["Wq"], np.float32).ravel(),
        np.asarray(inputs["Wk"], np.float32).ravel(),
        np.asarray(inputs["Wv"], np.float32).ravel(),
        np.asarray(inputs["Wp"], np.float32).ravel(),
        np.asarray(inputs["bq"], np.float32).ravel(),
        np.asarray(inputs["bk"], np.float32).ravel(),
        np.asarray(inputs["bv"], np.float32).ravel(),
        np.asarray(inputs["bp"], np.float32).ravel(),
        np.asarray(inputs["gamma"], np.float32).ravel(),
        np.asarray(inputs["beta"], np.float32).ravel(),
        np.eye(P, dtype=np.float32).ravel(),
        _egrp_const().ravel(),
        _egrpt_const().ravel(),
    ])
    in_maps = []
    for core in range(8):
        b, h = core // 2, core % 2
        if h == 0:
            xp = x_flat[b]
        else:
            xp = np.concatenate([x_flat[b, NQ:], x_flat[b, :NQ]], axis=0)
        # channel-major blob layout: x[cb, p, n] = xp[n, cb*128+p],
        # rounded to bf16 and packed into the first N*C/2 f32 words
        xp_cm = np.ascontiguousarray(xp.T.reshape(2, P, N))
        xp_bf = _to_bf16_bits(xp_cm)
        xp_packed = np.zeros(N * C, np.float32)
        xp_packed.view(np.uint16)[0:N * C] = xp_bf.ravel()
        in_maps.append({"blob": np.concatenate([xp_packed, shared])})
    return in_maps


def assemble(results: list[dict]) -> np.ndarray:
    y = np.empty((B, N, C), dtype=np.float32)
    for core in range(8):
        b, h = core // 2, core % 2
        o = results[core]["out"]  # [2, 128, NQ] channel-major
        y[b, h * NQ:(h + 1) * NQ] = o.reshape(C, NQ).T
    return y.reshape(B, H, W, C)


def kernel(**inputs) -> np.ndarray:
    fn = _get_fn()
    in_maps = make_in_maps(inputs)
    blob = np.concatenate([m["blob"] for m in in_maps])
    zeros = np.zeros((8 * 2, P, NQ), np.float32)
    (out,) = fn(blob, zeros)
    out = np.asarray(out).reshape(8, 2, P, NQ)
    return assemble([{"out": out[c]} for c in range(8)])
